# revision 3
# baseline (speedup 1.0000x reference)
"""Multi-head attention (16 heads, d_model=1024, B=2, S=2048) on 8 Trainium2
NeuronCores, tensor-parallel over heads (2 heads per core).

Per-core program (all matmuls bf16 with fp32 PSUM accumulation):
  - q_T/k_T = (W X^T + b) computed in transposed [d, token] layout.
    q is stored twice, zero-padded per head (q_pad0 has head-0 rows live and
    head-1 rows zero, q_pad1 vice versa) so each score matmul contracts over
    the full 128 partitions with the SHARED k stationary — the zeros kill the
    cross-head terms.  This keeps every matmul in the default 128x128 PE
    configuration (no 64-row tiling-mode switches, which drain the array) and
    makes the k ldweights FWL-eligible (full 128 columns).
  - v in natural [token, d] layout with a ones-column appended (gives the
    softmax denominators for free from the same attn@v matmul)
  - scores_T[j, q] = k-stationary matmul, exp on ScalarE straight out of
    PSUM (softmax without max-subtraction: scores ~ N(0,1), no overflow risk)
  - unnormalized attn output + denominators accumulate in PSUM; normalization
    applied during eviction via DVE reciprocal + Pool partition-broadcast +
    DVE multiply (no ScalarE work -> no activation-table thrash; the Exp
    table is loaded exactly once)
  - row block of Wo produces a partial [B*S, 1024] output per core; PSUM->SBUF
    evictions alternate DVE/Pool to keep both below the PE/ScalarE towers
Host: sum of the 8 partials + (bv @ Wo^T + bo) correction (exact because
softmax rows sum to 1, so the V-bias commutes out of attention).
"""

import os
import contextlib

import numpy as np
import ml_dtypes

import concourse.bass as bass
import concourse.tile as tile
import concourse.bacc as bacc
from concourse import mybir
from concourse import bass_utils

BF16 = ml_dtypes.bfloat16

D_MODEL = 1024
NUM_HEADS = 16
DK = 64
B, S = 2, 2048
BS = B * S
N_CORES = 8
HPC = NUM_HEADS // N_CORES          # heads per core = 2
DPC = HPC * DK                      # head-dim slice per core = 128
P = 128
NF = D_MODEL // P                   # 8 contraction tiles for projections
NIT = BS // P                       # 32 token tiles of 128
SJT = S // P                        # 16 key tiles per batch

f32 = mybir.dt.float32
bf16 = mybir.dt.bfloat16


def _env(name, default):
    return os.environ.get(name, default)


def _emit(tc, aps, loop=1):
    nc = tc.nc
    xq, xk, xv, wq, wk, wv, wo, bq, bk, out = aps
    QW = 512                       # attention query-chunk width
    NQC2 = S // QW                 # 4 chunks per batch
    KC0 = 512                      # first k column chunk loaded separately

    with contextlib.ExitStack() as ctx:
        const = ctx.enter_context(tc.tile_pool(name="const", bufs=1))
        xpool = ctx.enter_context(tc.tile_pool(
            name="xpool", bufs=int(_env("XB", "17"))))
        persist = ctx.enter_context(tc.tile_pool(name="persist", bufs=1))
        exp_pool = ctx.enter_context(tc.tile_pool(name="exp", bufs=8))
        attn_pool = ctx.enter_context(tc.tile_pool(name="attnp", bufs=6))
        bc_pool = ctx.enter_context(tc.tile_pool(name="bcast", bufs=3))
        rc_pool = ctx.enter_context(tc.tile_pool(name="recip", bufs=3))
        out_pool = ctx.enter_context(tc.tile_pool(name="outp", bufs=4))
        pp_pair = ctx.enter_context(tc.tile_pool(
            name="pp_pair", bufs=int(_env("PPPAIR", "2")), space="PSUM"))
        pp_av = ctx.enter_context(tc.tile_pool(
            name="pp_av", bufs=int(_env("PPAV", "3")), space="PSUM"))
        pp_blk = ctx.enter_context(tc.tile_pool(
            name="pp_blk", bufs=int(_env("PPBLK", "1")), space="PSUM"))

        # ---- constants (k-path first: it gates the first score matmuls) ----
        wk_sb = const.tile([P, NF, P], bf16)
        wq_sb = const.tile([P, NF, P], bf16)
        wv_sb = const.tile([P, NF, P], bf16)
        wo_sb = const.tile([P, D_MODEL], bf16)
        bq_sb = const.tile([P, 1], f32)
        bk_sb = const.tile([P, 1], f32)
        nc.sync.dma_start(wk_sb[:], wk.rearrange("(n p) m -> p n m", p=P))
        nc.sync.dma_start(bk_sb[:], bk[:])
        nc.sync.dma_start(wq_sb[:], wq.rearrange("(n p) m -> p n m", p=P))
        nc.sync.dma_start(bq_sb[:], bq[:])
        nc.sync.dma_start(wv_sb[:], wv.rearrange("(n p) m -> p n m", p=P))
        nc.sync.dma_start(wo_sb[:], wo[:])

        # q stored twice, zero-padded per head (see module docstring)
        q_pad = [persist.tile([P, BS], bf16, name=f"qpad{h}") for h in range(HPC)]
        k_sb = persist.tile([P, BS], bf16)
        v_sb = persist.tile([P, NIT, HPC * (DK + 1)], bf16)

        # one-time zero/one fills (outside the steady-state loop)
        nc.vector.memset(q_pad[0][DK : 2 * DK, :], 0.0)
        nc.vector.memset(q_pad[1][0:DK, :], 0.0)
        # ones columns of v_aug (softmax denominator rows)
        nc.vector.memset(v_sb[:, :, DK : DK + 1], 1.0)
        nc.vector.memset(v_sb[:, :, 2 * DK + 1 : 2 * DK + 2], 1.0)

        NOX = _env("NOX", "") == "1"
        NOPROJ = _env("NOPROJ", "") == "1"
        NOWO = _env("NOWO", "") == "1"
        if NOPROJ:
            for h in range(HPC):
                nc.vector.memset(q_pad[h][:], 0.1)
            nc.vector.memset(k_sb[:], 0.1)
            nc.vector.memset(v_sb[:], 0.1)

        loop_cm = tc.For_i(0, loop, 1) if loop > 1 else contextlib.nullcontext()
        with loop_cm:
            xt = {}

            def load_x(b):
                QWC = S // QW
                srcq = xq[:, b * S : (b + 1) * S].rearrange("(n p) m -> n p m", p=P)
                srck = xk[:, b * S : (b + 1) * S].rearrange("(n p) m -> n p m", p=P)
                srcv = xv[:, b * S : (b + 1) * S].rearrange("(n p) m -> n p m", p=P)
                # k, in two pieces: first KC0 columns gate the first chunk of
                # the k projection, so they get their own small DMAs
                ktiles = []
                for f in range(NF):
                    t = xpool.tile([P, S], bf16, tag="x")
                    nc.sync.dma_start(t[:, 0:KC0], srck[f, :, 0:KC0])
                    ktiles.append(t)
                for f in range(NF):
                    nc.sync.dma_start(ktiles[f][:, KC0:], srck[f, :, KC0:])
                xt[("k", b)] = ktiles
                # first q chunk right after k, before the bulk of v
                qtiles = [[None] * QWC for _ in range(NF)]
                xt[("q", b)] = qtiles
                for f in range(NF):
                    t = xpool.tile([P, QW], bf16, tag="xq",
                                   bufs=int(_env("XQB", "34")),
                                   name=f"xq{f}c0")
                    nc.sync.dma_start(t[:], srcq[f, :, 0:QW])
                    qtiles[f][0] = t
                vtiles = []
                for f in range(NF):
                    t = xpool.tile([P, S], bf16, tag="x")
                    nc.sync.dma_start(t[:], srcv[f])
                    vtiles.append(t)
                xt[("v", b)] = vtiles
                for c in range(1, QWC):
                    for f in range(NF):
                        t = xpool.tile([P, QW], bf16, tag="xq",
                                       bufs=int(_env("XQB", "34")),
                                       name=f"xq{f}c{c}")
                        nc.sync.dma_start(t[:], srcq[f, :, c * QW : (c + 1) * QW])
                        xt[("q", b)][f][c] = t

            def emit_qk(kind, b, c):
                ps = pp_blk.tile([P, QW], f32, tag="blk")
                cs = slice(c * QW, (c + 1) * QW)  # local within batch
                for f in range(NF):
                    rhs = (xt[(kind, b)][f][c][:]
                           if kind == "q" else xt[(kind, b)][f][:, cs])
                    nc.tensor.matmul(ps[:], (wq_sb if kind == "q" else wk_sb)[:, f, :],
                                     rhs, start=(f == 0), stop=(f == NF - 1))
                ds = slice(b * S + c * QW, b * S + (c + 1) * QW)
                if kind == "k":
                    nc.vector.tensor_scalar_add(k_sb[:, ds], ps[:], bk_sb[:])
                else:
                    # split per head into the two zero-padded q copies;
                    # h0 on DVE, h1 on ScalarE (Identity activation with
                    # per-partition bias) to balance the engines
                    nc.vector.tensor_scalar_add(
                        q_pad[0][0:DK, ds], ps[0:DK, :], bq_sb[0:DK])
                    nc.scalar.activation(
                        q_pad[1][DK : 2 * DK, ds], ps[DK : 2 * DK, :],
                        mybir.ActivationFunctionType.Identity,
                        bias=bq_sb[DK : 2 * DK])

            def emit_v(b, it2):
                ps = pp_blk.tile([P, QW], f32, tag="blk")
                isl = slice(it2 * P, (it2 + 1) * P)
                for f in range(NF):
                    nc.tensor.matmul(ps[:, 0:P], xt[("v", b)][f][:, isl],
                                     wv_sb[:, f, :],
                                     start=(f == 0), stop=(f == NF - 1))
                dst = v_sb[:, b * SJT + it2, 0:DK]
                dst = bass.AP(dst.tensor, dst.offset,
                              [dst.ap[0], [DK + 1, 2], [1, DK]])
                nc.vector.tensor_copy(dst, ps[:, 0:P].rearrange("p (a b) -> p a b", a=2))

            wo_evict_ctr = [0]

            def emit_wo(attn_c, b, qc, i2):
                po = pp_blk.tile([P, QW], f32, tag="blk")
                nc.tensor.matmul(po[:], attn_c[:, i2 * P : (i2 + 1) * P],
                                 wo_sb[:, 0:QW], start=True, stop=True)
                po2 = pp_blk.tile([P, QW], f32, tag="blk")
                nc.tensor.matmul(po2[:], attn_c[:, i2 * P : (i2 + 1) * P],
                                 wo_sb[:, QW:], start=True, stop=True)
                ot = out_pool.tile([P, D_MODEL], bf16)
                # Pool/GPSIMD cannot read PSUM, so evictions are split
                # DVE (3/4) / ScalarE (1/4) to keep DVE under the PE tower
                n = wo_evict_ctr[0]
                wo_evict_ctr[0] += 1
                if n % 2 == 0:
                    nc.vector.tensor_copy(ot[:, 0:QW], po[:])
                else:
                    nc.scalar.copy(ot[:, 0:QW], po[:])
                nc.vector.tensor_copy(ot[:, QW:], po2[:])
                row0 = b * S + qc * QW + i2 * P
                nc.sync.dma_start(out[row0 : row0 + P, :], ot[:])

            pending = []
            done = set()

            def emit_block(blk):
                key = blk[:1] + tuple(x for x in blk[1:] if not hasattr(x, "tensor"))
                if blk[0] == "qk_q":
                    emit_qk("q", blk[1], blk[2])
                elif blk[0] == "qk_k":
                    emit_qk("k", blk[1], blk[2])
                elif blk[0] == "v":
                    emit_v(blk[1], blk[2])
                else:
                    emit_wo(blk[1], blk[2], blk[3], blk[4])
                done.add(key)

            def force(key):
                if NOPROJ or key in done:
                    return
                for i, blk in enumerate(pending):
                    bkey = blk[:1] + tuple(x for x in blk[1:] if not hasattr(x, "tensor"))
                    if bkey == key:
                        pending.pop(i)
                        emit_block(blk)
                        return
                raise KeyError(key)

            def drain(n):
                for _ in range(min(n, len(pending))):
                    emit_block(pending.pop(0))

            for b in range(B):
                if not NOX:
                    load_x(b)
                if not NOPROJ:
                    for c in range(NQC2):
                        pending.append(("qk_k", b, c))
                    pending.append(("qk_q", b, 0))
                    for it2 in range(SJT):
                        pending.append(("v", b, it2))
                    for c in range(1, NQC2):
                        pending.append(("qk_q", b, c))

            if _env("PROJONLY", "") == "1":
                drain(len(pending))
                return
            SKEW = int(_env("SKEW", "3"))
            for b in range(B):
                # prologue for this batch: k fully, first q chunk
                for c in range(NQC2):
                    force(("qk_k", b, c))

                for qc in range(NQC2):
                    force(("qk_q", b, qc))
                    qss = slice(b * S + qc * QW, b * S + (qc + 1) * QW)
                    attn_c = attn_pool.tile([P, QW], bf16, tag="attn")
                    pav = [pp_av.tile([DK + 1, QW], f32, tag="av", name=f"pav{h}")
                           for h in range(HPC)]
                    ets = {}
                    for jt in range(SJT + SKEW):
                        if jt < SJT:
                            for la in range(SKEW + 1):
                                if jt + la < SJT:
                                    force(("v", b, jt + la))
                            jsl = slice(b * S + jt * P, b * S + (jt + 1) * P)
                            pair = pp_pair.tile([P, HPC, QW], f32, tag="pair")
                            for h in range(HPC):
                                # full-128 contraction: the zero rows of
                                # q_pad[h] cancel the other head's k rows
                                nc.tensor.matmul(
                                    pair[:, h, :], k_sb[:, jsl],
                                    q_pad[h][:, qss],
                                    start=True, stop=True,
                                )
                            et = exp_pool.tile([P, HPC, QW], bf16)
                            nc.scalar.activation(
                                et[:], pair[:],
                                mybir.ActivationFunctionType.Exp, scale=0.125,
                            )
                            ets[jt] = et
                        ja = jt - SKEW
                        if ja >= 0:
                            et = ets.pop(ja)
                            for h in range(HPC):
                                nc.tensor.matmul(
                                    pav[h][:],
                                    v_sb[:, b * SJT + ja,
                                         h * (DK + 1) : (h + 1) * (DK + 1)],
                                    et[:, h, :],
                                    start=(ja == 0), stop=(ja == SJT - 1),
                                )
                        drain(1)
                        if jt == 9:
                            nb, nqc = (b, qc + 1) if qc + 1 < NQC2 else (b + 1, 0)
                            if nb < B:
                                force(("qk_q", nb, nqc))
                    for h in range(HPC):
                        rc = rc_pool.tile([1, QW], f32)
                        nc.vector.reciprocal(rc[:], pav[h][DK : DK + 1, :])
                        bc = bc_pool.tile([DK, QW], f32)
                        nc.gpsimd.partition_broadcast(bc[:], rc[:])
                        nc.vector.tensor_mul(
                            attn_c[h * DK : (h + 1) * DK, :], pav[h][0:DK, :], bc[:])
                    if not NOWO:
                        for i2 in range(QW // P):
                            pending.append(("wo", attn_c, b, qc, i2))

            drain(len(pending))


def _build(loop=1):
    nc = bacc.Bacc("TRN2", target_bir_lowering=False, debug=False,
                   num_devices=N_CORES)
    xq = nc.dram_tensor("xq_t", [D_MODEL, BS], bf16, kind="ExternalInput").ap()
    xk = nc.dram_tensor("xk_t", [D_MODEL, BS], bf16, kind="ExternalInput").ap()
    xv = nc.dram_tensor("xv_t", [D_MODEL, BS], bf16, kind="ExternalInput").ap()
    wq = nc.dram_tensor("wq_t", [D_MODEL, DPC], bf16, kind="ExternalInput").ap()
    wk = nc.dram_tensor("wk_t", [D_MODEL, DPC], bf16, kind="ExternalInput").ap()
    wv = nc.dram_tensor("wv_t", [D_MODEL, DPC], bf16, kind="ExternalInput").ap()
    wo = nc.dram_tensor("wo_t", [DPC, D_MODEL], bf16, kind="ExternalInput").ap()
    bq = nc.dram_tensor("bq", [DPC, 1], f32, kind="ExternalInput").ap()
    bk = nc.dram_tensor("bk", [DPC, 1], f32, kind="ExternalInput").ap()
    out = nc.dram_tensor("out_p", [BS, D_MODEL], bf16, kind="ExternalOutput").ap()

    with tile.TileContext(nc) as tc:
        _emit(tc, (xq, xk, xv, wq, wk, wv, wo, bq, bk, out), loop=loop)
    nc.compile()
    return nc


_cache = {}


def _get_nc(loop=1):
    key = (loop,) + tuple(
        os.environ.get(k, "") for k in
        ("XB", "XQB", "SKEW", "PPPAIR", "PPAV", "PPBLK", "NOX", "NOPROJ",
         "NOWO", "PROJONLY"))
    if key not in _cache:
        _cache[key] = _build(loop)
    return _cache[key]


def _make_in_maps(Q, K, V, Wq, bq, Wk, bk, Wv, bv, Wo, bo):
    xq_t = np.ascontiguousarray(np.asarray(Q, np.float32).reshape(BS, D_MODEL).T).astype(BF16)
    xk_t = np.ascontiguousarray(np.asarray(K, np.float32).reshape(BS, D_MODEL).T).astype(BF16)
    xv_t = np.ascontiguousarray(np.asarray(V, np.float32).reshape(BS, D_MODEL).T).astype(BF16)
    in_maps = []
    for c in range(N_CORES):
        sl = slice(c * DPC, (c + 1) * DPC)
        in_maps.append({
            "xq_t": xq_t, "xk_t": xk_t, "xv_t": xv_t,
            "wq_t": np.ascontiguousarray(np.asarray(Wq)[sl].T).astype(BF16),
            "wk_t": np.ascontiguousarray(np.asarray(Wk)[sl].T).astype(BF16),
            "wv_t": np.ascontiguousarray(np.asarray(Wv)[sl].T).astype(BF16),
            "wo_t": np.ascontiguousarray(np.asarray(Wo)[:, sl].T).astype(BF16),
            "bq": np.asarray(bq, np.float32)[sl].reshape(DPC, 1).copy(),
            "bk": np.asarray(bk, np.float32)[sl].reshape(DPC, 1).copy(),
        })
    return in_maps


def kernel(Q, K, V, Wq, bq, Wk, bk, Wv, bv, Wo, bo):
    nc = _get_nc()
    in_maps = _make_in_maps(Q, K, V, Wq, bq, Wk, bk, Wv, bv, Wo, bo)
    res = bass_utils.run_bass_kernel_spmd(nc, in_maps, core_ids=list(range(N_CORES)))
    acc = np.zeros((BS, D_MODEL), np.float32)
    for c in range(N_CORES):
        acc += np.asarray(res.results[c]["out_p"], np.float32)
    corr = (np.asarray(bv, np.float64) @ np.asarray(Wo, np.float64).T
            + np.asarray(bo, np.float64)).astype(np.float32)
    return (acc + corr[None, :]).reshape(B, S, D_MODEL).astype(np.float32)
